# revision 1
# baseline (speedup 1.0000x reference)
"""Trainium2 Bass kernel for nn_Aspp_Attention: ASPP-KV attention over 2D features.

Sharding: pure data-parallel — batch b=8 over 8 NeuronCores, one image per core.
Device dataflow per core (x: (128 c, 16384 hw) f32):
  xp = x + pos  (pos DMA'd const)
  ASPP pools (hierarchical sums) -> depthwise 3x3 (DVE taps) -> pointwise (PE)
  LN (+transpose via PE) -> gelu(ln_w*z+ln_b) -> z2ct (c,85)
  A = [0.25*Wq_m^T Wk_m] z2ct  (c,680)     [weights folded host-side]
  scores^T chunks (kv-major, 6 chunks of <=128 rows) = A_chunk^T @ xp  (PE)
  exp on ACT (PSUM->SBUF), softmax denominators r via ones-block matmuls (PE)
  H = blockdiag(v)^T @ exp  (PE, per-head value mix), Hn = H * (1/r) (DVE)
  out^T = Wproj @ Hn + bproj (PE + DVE), DMA out.
"""
import os
from contextlib import ExitStack

import numpy as np

B, C, Hh, Ww = 8, 128, 128, 128
HW = Hh * Ww
M, HD, KV = 8, 16, 85
KVH = M * KV  # 680
CH_B = [0, 128, 256, 384, 512, 640, 680]
NCH = 6
G = 512            # token group
NG = HW // G       # 32

_CACHE = {}


def _pos_full():
    ch = 64
    inv = 1.0 / (10000.0 ** (np.arange(0, ch, 2, dtype=np.float32) / ch))
    px = np.arange(Hh, dtype=np.float32)[:, None] * inv
    ex = np.concatenate([np.sin(px), np.cos(px)], -1).astype(np.float32)  # (128,64)
    pos = np.zeros((C, Hh, Ww), np.float32)
    pos[:64] = ex.T[:, :, None]
    pos[64:] = ex.T[:, None, :]
    return pos.reshape(C, HW)


def _build():
    import concourse.bass as bass
    import concourse.bacc as bacc
    import concourse.tile as tile
    from concourse import mybir

    nc = bacc.Bacc()
    f32 = mybir.dt.float32
    AF = mybir.ActivationFunctionType
    AX = mybir.AxisListType

    x_d = nc.dram_tensor("x", [C, HW], f32, kind="ExternalInput")
    pos_d = nc.dram_tensor("pos", [C, HW], f32, kind="ExternalInput")
    ct_d = nc.dram_tensor("ct", [M * C, C], f32, kind="ExternalInput")      # lhsT for A
    wvt_d = nc.dram_tensor("wvt", [C, C], f32, kind="ExternalInput")
    pwt_d = nc.dram_tensor("pwt", [C, C], f32, kind="ExternalInput")
    wpt_d = nc.dram_tensor("wpt", [C, C], f32, kind="ExternalInput")
    dwdiag_d = nc.dram_tensor("dwdiag", [C, 36 * C], f32, kind="ExternalInput")  # diag taps
    ones_d = nc.dram_tensor("onesb", [C, NCH * M], f32, kind="ExternalInput")
    lnw_d = nc.dram_tensor("lnw", [C, 1], f32, kind="ExternalInput")
    lnb_d = nc.dram_tensor("lnb", [C, 1], f32, kind="ExternalInput")
    bpj_d = nc.dram_tensor("bpj", [C, 1], f32, kind="ExternalInput")
    idn_d = nc.dram_tensor("idn", [C, C], f32, kind="ExternalInput")
    erep_d = nc.dram_tensor("erep", [M, C], f32, kind="ExternalInput")
    out_d = nc.dram_tensor("out", [C, HW], f32, kind="ExternalOutput")

    with ExitStack() as ctx:
        tc = ctx.enter_context(tile.TileContext(nc))
        singles = ctx.enter_context(tc.tile_pool(name="singles", bufs=1))
        xpool = ctx.enter_context(tc.tile_pool(name="xp", bufs=1))
        small = ctx.enter_context(tc.tile_pool(name="small", bufs=2))
        exp_pool = ctx.enter_context(tc.tile_pool(name="exp", bufs=2))
        outp = ctx.enter_context(tc.tile_pool(name="outs", bufs=3))
        rr_pool = ctx.enter_context(tc.tile_pool(name="rr", bufs=3))
        ps_big = ctx.enter_context(tc.tile_pool(name="psA", bufs=1, space="PSUM"))
        ps_b2 = ctx.enter_context(tc.tile_pool(name="psB", bufs=1, space="PSUM"))
        ps_h = ctx.enter_context(tc.tile_pool(name="psH", bufs=1, space="PSUM"))
        ps_ro = ctx.enter_context(tc.tile_pool(name="psRO", bufs=1, space="PSUM"))

        # ---- load constants
        ct_sb = singles.tile([C, M * C], f32)       # ct_sb[:, m*C:(m+1)*C] = CT_m
        for m in range(M):
            nc.gpsimd.dma_start(out=ct_sb[:, m * C:(m + 1) * C],
                              in_=ct_d[m * C:(m + 1) * C, :])
        wvt_sb = singles.tile([C, C], f32)
        nc.gpsimd.dma_start(out=wvt_sb, in_=wvt_d[:, :])
        pwt_sb = singles.tile([C, C], f32)
        nc.gpsimd.dma_start(out=pwt_sb, in_=pwt_d[:, :])
        wpt_sb = singles.tile([C, C], f32)
        nc.gpsimd.dma_start(out=wpt_sb, in_=wpt_d[:, :])
        dwdiag_sb = singles.tile([C, 36 * C], f32)
        nc.gpsimd.dma_start(out=dwdiag_sb, in_=dwdiag_d[:, :])
        ones_sb = singles.tile([C, NCH * M], f32)
        nc.gpsimd.dma_start(out=ones_sb, in_=ones_d[:, :])
        lnw_sb = singles.tile([C, 1], f32)
        nc.gpsimd.dma_start(out=lnw_sb, in_=lnw_d[:, :])
        lnb_sb = singles.tile([C, 1], f32)
        nc.gpsimd.dma_start(out=lnb_sb, in_=lnb_d[:, :])
        bpj_sb = singles.tile([C, 1], f32)
        nc.gpsimd.dma_start(out=bpj_sb, in_=bpj_d[:, :])
        idn_sb = singles.tile([C, C], f32)
        nc.gpsimd.dma_start(out=idn_sb, in_=idn_d[:, :])
        erep_sb = singles.tile([M, C], f32)
        nc.gpsimd.dma_start(out=erep_sb, in_=erep_d[:, :])

        # ---- x + pos -> xp (chunked so DMA/DVE pipeline)
        xp_sb = xpool.tile([C, HW], f32)
        NXC = 8
        xc = HW // NXC
        for i in range(NXC):
            xt = small.tile([C, xc], f32, tag="xin")
            pt = small.tile([C, xc], f32, tag="pin")
            nc.gpsimd.dma_start(out=xt, in_=x_d[:, i * xc:(i + 1) * xc])
            nc.gpsimd.dma_start(out=pt, in_=pos_d[:, i * xc:(i + 1) * xc])
            nc.vector.tensor_add(xp_sb[:, i * xc:(i + 1) * xc], xt, pt)

        # ---- pools (sums)
        xp3 = xp_sb.rearrange("c (h w) -> c h w", h=Hh)
        s1 = singles.tile([C, Hh, 8], f32)   # sum over w-blocks of 16
        nc.vector.reduce_sum(
            s1, xp3.rearrange("c h (wg wi) -> c h wg wi", wi=16), axis=AX.X)
        p8 = singles.tile([C, 8, 8], f32)
        nc.vector.reduce_sum(
            p8, s1.rearrange("c (hg hi) wg -> c hg wg hi", hi=16), axis=AX.X)
        p4 = singles.tile([C, 4, 4], f32)
        t44 = singles.tile([C, 8, 4], f32)
        nc.vector.reduce_sum(t44, p8.rearrange("c h (wg wi) -> c h wg wi", wi=2), axis=AX.X)
        nc.vector.reduce_sum(p4, t44.rearrange("c (hg hi) w -> c hg w hi", hi=2), axis=AX.X)
        p2 = singles.tile([C, 2, 2], f32)
        t22 = singles.tile([C, 4, 2], f32)
        nc.vector.reduce_sum(t22, p4.rearrange("c h (wg wi) -> c h wg wi", wi=2), axis=AX.X)
        nc.vector.reduce_sum(p2, t22.rearrange("c (hg hi) w -> c hg w hi", hi=2), axis=AX.X)
        p1 = singles.tile([C, 1, 1], f32)
        t11 = singles.tile([C, 2, 1], f32)
        nc.vector.reduce_sum(t11, p2.rearrange("c h (wg wi) -> c h wg wi", wi=2), axis=AX.X)
        nc.vector.reduce_sum(p1, t11.rearrange("c (hg hi) w -> c hg w hi", hi=2), axis=AX.X)

        # ---- depthwise 3x3 on each level (padded, 9 taps) -> dwcat (C, 85)
        dwcat = singles.tile([C, KV], f32)
        offs = {8: 0, 4: 64, 2: 80, 1: 84}
        for lvl, (s, ps) in enumerate(((8, p8), (4, p4), (2, p2), (1, p1))):
            pad = singles.tile([C, (s + 2) * (s + 2)], f32, tag=f"pad{s}")
            nc.vector.memset(pad, 0.0)
            pad3 = pad.rearrange("c (h w) -> c h w", h=s + 2)
            nc.vector.tensor_copy(pad3[:, 1:s + 1, 1:s + 1], ps)
            o = offs[s]
            acc_ps = ps_ro.tile([C, s * s], f32, tag="ro")
            for di in range(3):
                for dj in range(3):
                    t = 3 * di + dj
                    dg = dwdiag_sb[:, (lvl * 9 + t) * C:(lvl * 9 + t + 1) * C]
                    src = pad3[:, di:di + s, dj:dj + s]
                    nc.tensor.matmul(acc_ps.rearrange("c (h w) -> c h w", h=s), lhsT=dg,
                                     rhs=src, start=(t == 0), stop=(t == 8))
            nc.scalar.copy(dwcat[:, o:o + s * s], acc_ps)

        # ---- pointwise conv (PE): z1 = PW @ dwcat
        z1_ps = ps_ro.tile([C, KV], f32, tag="ro")
        nc.tensor.matmul(z1_ps, lhsT=pwt_sb, rhs=dwcat, start=True, stop=True)
        z1_sb = singles.tile([C, KV], f32)
        nc.scalar.copy(z1_sb, z1_ps)

        # ---- LN over c: transpose -> stats -> zn -> transpose back -> gelu
        zt_ps = ps_ro.tile([KV, C], f32, tag="ro")
        nc.tensor.transpose(zt_ps, z1_sb, idn_sb)
        zt_sb = singles.tile([KV, C], f32)
        nc.scalar.copy(zt_sb, zt_ps)
        nmu = singles.tile([KV, 1], f32)
        nc.vector.reduce_sum(nmu, zt_sb, axis=AX.X, negate=True)
        nc.vector.tensor_scalar_mul(nmu, nmu, 1.0 / C)
        zc = singles.tile([KV, C], f32)
        nc.vector.tensor_scalar_add(zc, zt_sb, nmu)
        sq = singles.tile([KV, C], f32)
        nc.vector.tensor_mul(sq, zc, zc)
        var = singles.tile([KV, 1], f32)
        nc.vector.reduce_sum(var, sq, axis=AX.X)
        std = singles.tile([KV, 1], f32)
        eps_sb = singles.tile([KV, 1], f32)
        nc.vector.memset(eps_sb, 1e-5)
        nc.scalar.activation(std, var, AF.Sqrt, bias=eps_sb, scale=1.0 / C)
        rstd = singles.tile([KV, 1], f32)
        nc.vector.reciprocal(rstd, std)
        zn = singles.tile([KV, C], f32)
        nc.vector.tensor_scalar_mul(zn, zc, rstd)
        znt_ps = ps_ro.tile([C, KV], f32, tag="ro")
        nc.tensor.transpose(znt_ps, zn, idn_sb[:KV, :KV])
        z2 = singles.tile([C, KV], f32)
        nc.scalar.activation(z2, znt_ps, AF.Gelu, bias=lnb_sb, scale=lnw_sb)

        # ---- A (c, 680), vkv (85, 128), B2 chunks (c-part rows are kv)
        a_sb = singles.tile([C, KVH], f32)
        for half in range(2):
            a_ps = ps_ro.tile([C, 4 * KV], f32, tag="ro")
            for mi in range(4):
                m = half * 4 + mi
                nc.tensor.matmul(a_ps[:, mi * KV:(mi + 1) * KV],
                                 lhsT=ct_sb[:, m * C:(m + 1) * C], rhs=z2,
                                 start=True, stop=True)
            nc.scalar.copy(a_sb[:, half * 4 * KV:(half + 1) * 4 * KV], a_ps)
        vt_ps = ps_ro.tile([C, KV], f32, tag="ro")
        nc.tensor.matmul(vt_ps, lhsT=wvt_sb, rhs=z2, start=True, stop=True)
        vt_sb = singles.tile([C, KV], f32)
        nc.scalar.copy(vt_sb, vt_ps)
        vkv_ps = ps_ro.tile([KV, C], f32, tag="ro")
        nc.tensor.transpose(vkv_ps, vt_sb, idn_sb)
        vkv_sb = singles.tile([KV, C], f32)
        nc.scalar.copy(vkv_sb, vkv_ps)

        b2_sb = singles.tile([C, NCH * C], f32)
        nc.vector.memset(b2_sb, 0.0)
        for m in range(M):
            g0, g1 = KV * m, KV * (m + 1)
            for cchunk in range(NCH):
                c0, c1 = CH_B[cchunk], CH_B[cchunk + 1]
                lo, hi = max(g0, c0), min(g1, c1)
                if lo >= hi:
                    continue
                nc.gpsimd.dma_start(
                    out=b2_sb[lo - c0:hi - c0,
                              cchunk * C + HD * m: cchunk * C + HD * m + HD],
                    in_=vkv_sb[lo - g0:hi - g0, HD * m:HD * m + HD])

        # ---- main attention loop over token groups
        for g in range(NG):
            t0 = g * G
            xg = xp_sb[:, t0:t0 + G]
            sA = ps_big.tile([C, 4 * G], f32)     # chunks 0-3
            sB = ps_b2.tile([C, 2 * G], f32)      # chunks 4-5
            for cc in range(NCH):
                c0, c1 = CH_B[cc], CH_B[cc + 1]
                lhs = a_sb[:, c0:c1]
                if cc < 4:
                    dst = sA[:c1 - c0, cc * G:(cc + 1) * G]
                else:
                    dst = sB[:c1 - c0, (cc - 4) * G:(cc - 3) * G]
                nc.tensor.matmul(dst, lhsT=lhs, rhs=xg, start=True, stop=True)
            ex_sb = exp_pool.tile([C, NCH * G], f32)
            nc.scalar.activation(ex_sb[:, :4 * G], sA, AF.Exp)
            nc.scalar.activation(ex_sb[:, 4 * G:], sB, AF.Exp)

            h_ps = ps_h.tile([C, G], f32)
            r_ps = ps_ro.tile([M, G], f32, tag="ro")
            for cc in range(NCH):
                k = CH_B[cc + 1] - CH_B[cc]
                eslice = ex_sb[:k, cc * G:(cc + 1) * G]
                nc.tensor.matmul(h_ps, lhsT=b2_sb[:k, cc * C:cc * C + C],
                                 rhs=eslice, start=(cc == 0), stop=(cc == NCH - 1))
                nc.tensor.matmul(r_ps, lhsT=ones_sb[:k, cc * M:(cc + 1) * M],
                                 rhs=eslice, start=(cc == 0), stop=(cc == NCH - 1))
            rec = rr_pool.tile([M, G], f32, tag="rec")
            nc.vector.reciprocal_approx_fast(rec, r_ps)
            rrep_ps = ps_ro.tile([C, G], f32, tag="ro")
            nc.tensor.matmul(rrep_ps, lhsT=erep_sb, rhs=rec, start=True, stop=True)
            rrep = rr_pool.tile([C, G], f32, tag="rrep")
            nc.vector.tensor_copy(rrep, rrep_ps)
            hn = rr_pool.tile([C, G], f32, tag="hn")
            nc.vector.tensor_mul(hn, h_ps, rrep)
            o_ps = ps_ro.tile([C, G], f32, tag="ro")
            nc.tensor.matmul(o_ps, lhsT=wpt_sb, rhs=hn, start=True, stop=True)
            o_sb = outp.tile([C, G], f32)
            nc.vector.tensor_scalar_add(o_sb, o_ps, bpj_sb)
            nc.gpsimd.dma_start(out=out_d[:, t0:t0 + G], in_=o_sb)

    nc.finalize()
    return nc


def _consts(Wq, Wkv, Wproj, bproj, dw_w, pw_w, ln_w, ln_b):
    scale = HD ** -0.5
    Wk, Wv = Wkv[:128], Wkv[128:]
    ct = np.zeros((M * C, C), np.float32)
    for m in range(M):
        ct[m * C:(m + 1) * C] = scale * Wk[16 * m:16 * m + 16].T @ Wq[16 * m:16 * m + 16]
    dwdiag = np.zeros((C, 36 * C), np.float32)
    for lvl, s in enumerate((8, 4, 2, 1)):
        lscale = (s * s) / float(HW)
        taps = dw_w[:, 0].reshape(C, 9) * lscale
        for t in range(9):
            i = lvl * 9 + t
            dwdiag[:, i * C:(i + 1) * C] = np.diag(taps[:, t])
    onesb = np.zeros((C, NCH * M), np.float32)
    for cc in range(NCH):
        c0, c1 = CH_B[cc], CH_B[cc + 1]
        for r in range(c1 - c0):
            onesb[r, cc * M + (c0 + r) // KV] = 1.0
    return {
        "ct": ct,
        "wvt": np.ascontiguousarray(Wv.T),
        "pwt": np.ascontiguousarray(pw_w[:, :, 0, 0].T),
        "wpt": np.ascontiguousarray(Wproj.T),
        "dwdiag": dwdiag,
        "onesb": onesb,
        "lnw": ln_w.reshape(C, 1).astype(np.float32),
        "lnb": ln_b.reshape(C, 1).astype(np.float32),
        "bpj": bproj.reshape(C, 1).astype(np.float32),
        "idn": np.eye(C, dtype=np.float32),
        "erep": np.kron(np.eye(M, dtype=np.float32), np.ones((1, HD), np.float32)),
        "pos": _pos_full(),
    }


def kernel(x, Wq, Wkv, Wproj, bproj, dw_w, pw_w, ln_w, ln_b):
    from concourse.bass_utils import run_bass_kernel_spmd

    if "nc" not in _CACHE:
        _CACHE["nc"] = _build()
    nc = _CACHE["nc"]

    cst = _consts(np.asarray(Wq, np.float32), np.asarray(Wkv, np.float32),
                  np.asarray(Wproj, np.float32), np.asarray(bproj, np.float32),
                  np.asarray(dw_w, np.float32), np.asarray(pw_w, np.float32),
                  np.asarray(ln_w, np.float32), np.asarray(ln_b, np.float32))
    x = np.asarray(x, np.float32)
    in_maps = []
    for b in range(B):
        im = {"x": np.ascontiguousarray(x[b].reshape(C, HW))}
        im.update(cst)
        in_maps.append(im)

    trace = bool(int(os.environ.get("KPROF", "0")))
    res = run_bass_kernel_spmd(nc, in_maps, core_ids=list(range(B)), trace=trace)
    if trace and res.exec_time_ns is not None:
        print(f"HW exec time: {res.exec_time_ns} ns")
    out = np.stack([res.results[b]["out"].reshape(C, Hh, Ww) for b in range(B)])
    return out



# revision 22
# speedup vs baseline: 2.0803x; 2.0803x over previous
"""Trainium2 Bass kernel for nn_Aspp_Attention: ASPP-KV attention over 2D features.

Sharding: pure data-parallel — batch b=8 over 8 NeuronCores, one image per core.
Device dataflow per core (x: (128 c, 16384 hw) f32):
  xp = x + pos  (pos DMA'd const)
  ASPP pools (hierarchical sums) -> depthwise 3x3 (DVE taps) -> pointwise (PE)
  LN (+transpose via PE) -> gelu(ln_w*z+ln_b) -> z2ct (c,85)
  A = [0.25*Wq_m^T Wk_m] z2ct  (c,680)     [weights folded host-side]
  scores^T chunks (kv-major, 6 chunks of <=128 rows) = A_chunk^T @ xp  (PE)
  exp on ACT (PSUM->SBUF), softmax denominators r via ones-block matmuls (PE)
  H = blockdiag(v)^T @ exp  (PE, per-head value mix), Hn = H * (1/r) (DVE)
  out^T = Wproj @ Hn + bproj (PE + DVE), DMA out.
"""
import os
from contextlib import ExitStack

import numpy as np

B, C, Hh, Ww = 8, 128, 128, 128
HW = Hh * Ww
M, HD, KV = 8, 16, 85
KVH = M * KV  # 680
CH_B = [0, 128, 256, 384, 512, 640, 680]
NCH = 6
G = 512            # token group
NG = HW // G       # 32

_CACHE = {}


def _pos_full():
    ch = 64
    inv = 1.0 / (10000.0 ** (np.arange(0, ch, 2, dtype=np.float32) / ch))
    px = np.arange(Hh, dtype=np.float32)[:, None] * inv
    ex = np.concatenate([np.sin(px), np.cos(px)], -1).astype(np.float32)  # (128,64)
    pos = np.zeros((C, Hh, Ww), np.float32)
    pos[:64] = ex.T[:, :, None]
    pos[64:] = ex.T[:, None, :]
    return pos.reshape(C, HW)


def _build():
    import concourse.bass as bass
    import concourse.bacc as bacc
    import concourse.tile as tile
    from concourse import mybir

    nc = bacc.Bacc()
    f32 = mybir.dt.float32
    bf16 = mybir.dt.bfloat16
    AF = mybir.ActivationFunctionType
    AX = mybir.AxisListType

    x_d = nc.dram_tensor("x", [C, HW], f32, kind="ExternalInput")
    pos_d = nc.dram_tensor("pos", [C, HW], f32, kind="ExternalInput")
    ct_d = nc.dram_tensor("ct", [M * C, C], f32, kind="ExternalInput")      # lhsT for A
    wvt_d = nc.dram_tensor("wvt", [C, C], f32, kind="ExternalInput")
    pwt_d = nc.dram_tensor("pwt", [C, C], f32, kind="ExternalInput")
    wpt_d = nc.dram_tensor("wpt", [C, C], bf16, kind="ExternalInput")
    dwdiag_d = nc.dram_tensor("dwdiag", [C, 36 * C], f32, kind="ExternalInput")  # diag taps
    ones_d = nc.dram_tensor("onesb", [C, NCH * M], bf16, kind="ExternalInput")
    lnw_d = nc.dram_tensor("lnw", [C, 1], f32, kind="ExternalInput")
    lnb_d = nc.dram_tensor("lnb", [C, 1], f32, kind="ExternalInput")
    bpj_d = nc.dram_tensor("bpj", [C, 1], f32, kind="ExternalInput")
    idn_d = nc.dram_tensor("idn", [C, C], f32, kind="ExternalInput")
    erep_d = nc.dram_tensor("erep", [M, C], bf16, kind="ExternalInput")
    out_d = nc.dram_tensor("out", [C, HW], f32, kind="ExternalOutput")

    with ExitStack() as ctx:
        tc = ctx.enter_context(tile.TileContext(nc))
        singles = ctx.enter_context(tc.tile_pool(name="singles", bufs=1))
        xpool = ctx.enter_context(tc.tile_pool(name="xp", bufs=1))
        small = ctx.enter_context(tc.tile_pool(name="small", bufs=2))
        exp_pool = ctx.enter_context(tc.tile_pool(name="exp", bufs=2))
        outp = ctx.enter_context(tc.tile_pool(name="outs", bufs=3))
        rr_pool = ctx.enter_context(tc.tile_pool(name="rr", bufs=3))
        ps_big = ctx.enter_context(tc.tile_pool(name="psA", bufs=1, space="PSUM"))
        ps_b2 = ctx.enter_context(tc.tile_pool(name="psB", bufs=1, space="PSUM"))
        ps_h = ctx.enter_context(tc.tile_pool(name="psH", bufs=1, space="PSUM"))
        ps_ro = ctx.enter_context(tc.tile_pool(name="psRO", bufs=1, space="PSUM"))

        # ---- load constants
        ct_sb = singles.tile([C, M * C], f32)       # ct_sb[:, m*C:(m+1)*C] = CT_m
        for m in range(M):
            nc.gpsimd.dma_start(out=ct_sb[:, m * C:(m + 1) * C],
                              in_=ct_d[m * C:(m + 1) * C, :])
        wvt_sb = singles.tile([C, C], f32)
        nc.gpsimd.dma_start(out=wvt_sb, in_=wvt_d[:, :])
        pwt_sb = singles.tile([C, C], f32)
        nc.gpsimd.dma_start(out=pwt_sb, in_=pwt_d[:, :])
        wpt_sb = singles.tile([C, C], bf16)
        nc.gpsimd.dma_start(out=wpt_sb, in_=wpt_d[:, :])
        dwdiag_sb = singles.tile([C, 36 * C], f32)
        nc.gpsimd.dma_start(out=dwdiag_sb, in_=dwdiag_d[:, :])
        ones_sb = singles.tile([C, NCH * M], bf16)
        nc.gpsimd.dma_start(out=ones_sb, in_=ones_d[:, :])
        lnw_sb = singles.tile([C, 1], f32)
        nc.gpsimd.dma_start(out=lnw_sb, in_=lnw_d[:, :])
        lnb_sb = singles.tile([C, 1], f32)
        nc.gpsimd.dma_start(out=lnb_sb, in_=lnb_d[:, :])
        bpj_sb = singles.tile([C, 1], f32)
        nc.gpsimd.dma_start(out=bpj_sb, in_=bpj_d[:, :])
        idn_sb = singles.tile([C, C], f32)
        nc.gpsimd.dma_start(out=idn_sb, in_=idn_d[:, :])
        erep_sb = singles.tile([M, C], bf16)
        nc.gpsimd.dma_start(out=erep_sb, in_=erep_d[:, :])

        # ---- x + pos -> xp (chunked so DMA/DVE pipeline)
        xp_sb = xpool.tile([C, HW], bf16)
        NXC = 8
        xc = HW // NXC
        for i in range(NXC):
            xt = small.tile([C, xc], f32, tag="xin")
            pt = small.tile([C, xc], f32, tag="pin")
            nc.gpsimd.dma_start(out=xt, in_=x_d[:, i * xc:(i + 1) * xc])
            nc.gpsimd.dma_start(out=pt, in_=pos_d[:, i * xc:(i + 1) * xc])
            nc.vector.tensor_add(xp_sb[:, i * xc:(i + 1) * xc], xt, pt)

        # ---- pools (sums)
        xp3 = xp_sb.rearrange("c (h w) -> c h w", h=Hh)
        s1 = singles.tile([C, Hh, 8], f32)   # sum over w-blocks of 16
        nc.vector.reduce_sum(
            s1, xp3.rearrange("c h (wg wi) -> c h wg wi", wi=16), axis=AX.X)
        p8 = singles.tile([C, 8, 8], f32)
        nc.vector.reduce_sum(
            p8, s1.rearrange("c (hg hi) wg -> c hg wg hi", hi=16), axis=AX.X)
        p4 = singles.tile([C, 4, 4], f32)
        t44 = singles.tile([C, 8, 4], f32)
        nc.vector.reduce_sum(t44, p8.rearrange("c h (wg wi) -> c h wg wi", wi=2), axis=AX.X)
        nc.vector.reduce_sum(p4, t44.rearrange("c (hg hi) w -> c hg w hi", hi=2), axis=AX.X)
        p2 = singles.tile([C, 2, 2], f32)
        t22 = singles.tile([C, 4, 2], f32)
        nc.vector.reduce_sum(t22, p4.rearrange("c h (wg wi) -> c h wg wi", wi=2), axis=AX.X)
        nc.vector.reduce_sum(p2, t22.rearrange("c (hg hi) w -> c hg w hi", hi=2), axis=AX.X)
        p1 = singles.tile([C, 1, 1], f32)
        t11 = singles.tile([C, 2, 1], f32)
        nc.vector.reduce_sum(t11, p2.rearrange("c h (wg wi) -> c h wg wi", wi=2), axis=AX.X)
        nc.vector.reduce_sum(p1, t11.rearrange("c (hg hi) w -> c hg w hi", hi=2), axis=AX.X)

        # ---- depthwise 3x3 on each level (padded, 9 taps) -> dwcat (C, 85)
        dwcat = singles.tile([C, KV], f32)
        offs = {8: 0, 4: 64, 2: 80, 1: 84}
        for lvl, (s, ps) in enumerate(((8, p8), (4, p4), (2, p2), (1, p1))):
            pad = singles.tile([C, (s + 2) * (s + 2)], f32, tag=f"pad{s}")
            nc.vector.memset(pad, 0.0)
            pad3 = pad.rearrange("c (h w) -> c h w", h=s + 2)
            nc.vector.tensor_copy(pad3[:, 1:s + 1, 1:s + 1], ps)
            o = offs[s]
            acc_ps = ps_ro.tile([C, s * s], f32, tag="ro")
            for di in range(3):
                for dj in range(3):
                    t = 3 * di + dj
                    dg = dwdiag_sb[:, (lvl * 9 + t) * C:(lvl * 9 + t + 1) * C]
                    src = pad3[:, di:di + s, dj:dj + s]
                    nc.tensor.matmul(acc_ps.rearrange("c (h w) -> c h w", h=s), lhsT=dg,
                                     rhs=src, start=(t == 0), stop=(t == 8))
            nc.scalar.copy(dwcat[:, o:o + s * s], acc_ps)

        # ---- pointwise conv (PE): z1 = PW @ dwcat
        z1_ps = ps_ro.tile([C, KV], f32, tag="ro")
        nc.tensor.matmul(z1_ps, lhsT=pwt_sb, rhs=dwcat, start=True, stop=True)
        z1_sb = singles.tile([C, KV], f32)
        nc.scalar.copy(z1_sb, z1_ps)

        # ---- LN over c: transpose -> stats -> zn -> transpose back -> gelu
        zt_ps = ps_ro.tile([KV, C], f32, tag="ro")
        nc.tensor.transpose(zt_ps, z1_sb, idn_sb)
        zt_sb = singles.tile([KV, C], f32)
        nc.scalar.copy(zt_sb, zt_ps)
        nmu = singles.tile([KV, 1], f32)
        nc.vector.reduce_sum(nmu, zt_sb, axis=AX.X, negate=True)
        nc.vector.tensor_scalar_mul(nmu, nmu, 1.0 / C)
        zc = singles.tile([KV, C], f32)
        nc.vector.tensor_scalar_add(zc, zt_sb, nmu)
        sq = singles.tile([KV, C], f32)
        nc.vector.tensor_mul(sq, zc, zc)
        var = singles.tile([KV, 1], f32)
        nc.vector.reduce_sum(var, sq, axis=AX.X)
        std = singles.tile([KV, 1], f32)
        eps_sb = singles.tile([KV, 1], f32)
        nc.vector.memset(eps_sb, 1e-5)
        nc.scalar.activation(std, var, AF.Sqrt, bias=eps_sb, scale=1.0 / C)
        rstd = singles.tile([KV, 1], f32)
        nc.vector.reciprocal(rstd, std)
        zn = singles.tile([KV, C], f32)
        nc.vector.tensor_scalar_mul(zn, zc, rstd)
        znt_ps = ps_ro.tile([C, KV], f32, tag="ro")
        nc.tensor.transpose(znt_ps, zn, idn_sb[:KV, :KV])
        z2 = singles.tile([C, KV], f32)
        nc.scalar.activation(z2, znt_ps, AF.Gelu, bias=lnb_sb, scale=lnw_sb)

        # ---- A (c, 680), vkv (85, 128), B2 chunks (c-part rows are kv)
        a_sb = singles.tile([C, KVH], bf16)
        for half in range(2):
            a_ps = ps_ro.tile([C, 4 * KV], f32, tag="ro")
            for mi in range(4):
                m = half * 4 + mi
                nc.tensor.matmul(a_ps[:, mi * KV:(mi + 1) * KV],
                                 lhsT=ct_sb[:, m * C:(m + 1) * C], rhs=z2,
                                 start=True, stop=True)
            nc.scalar.copy(a_sb[:, half * 4 * KV:(half + 1) * 4 * KV], a_ps)
        vt_ps = ps_ro.tile([C, KV], f32, tag="ro")
        nc.tensor.matmul(vt_ps, lhsT=wvt_sb, rhs=z2, start=True, stop=True)
        vt_sb = singles.tile([C, KV], f32)
        nc.scalar.copy(vt_sb, vt_ps)
        vkv_ps = ps_ro.tile([KV, C], f32, tag="ro")
        nc.tensor.transpose(vkv_ps, vt_sb, idn_sb)
        vkv_sb = singles.tile([KV, C], bf16)
        nc.scalar.copy(vkv_sb, vkv_ps)

        b2_sb = singles.tile([C, NCH * C], bf16)
        nc.vector.memset(b2_sb, 0.0)
        for m in range(M):
            g0, g1 = KV * m, KV * (m + 1)
            for cchunk in range(NCH):
                c0, c1 = CH_B[cchunk], CH_B[cchunk + 1]
                lo, hi = max(g0, c0), min(g1, c1)
                if lo >= hi:
                    continue
                nc.gpsimd.dma_start(
                    out=b2_sb[lo - c0:hi - c0,
                              cchunk * C + HD * m: cchunk * C + HD * m + HD],
                    in_=vkv_sb[lo - g0:hi - g0, HD * m:HD * m + HD])

        # ---- main attention loop over token groups
        for g in range(NG):
            t0 = g * G
            xg = xp_sb[:, t0:t0 + G]
            sA = ps_big.tile([C, 4 * G], f32)     # chunks 0-3
            sB = ps_b2.tile([C, 2 * G], f32)      # chunks 4-5
            for cc in range(NCH):
                c0, c1 = CH_B[cc], CH_B[cc + 1]
                if cc < 4:
                    dst = sA[:c1 - c0, cc * G:(cc + 1) * G]
                else:
                    dst = sB[:c1 - c0, (cc - 4) * G:(cc - 3) * G]
                nc.tensor.matmul(dst, lhsT=a_sb[:, c0:c1], rhs=xg,
                                 start=True, stop=True)
            ex_sb = exp_pool.tile([C, NCH * G], bf16)
            nc.scalar.activation(ex_sb[:, :4 * G], sA, AF.Exp)
            nc.scalar.activation(ex_sb[:, 4 * G:], sB, AF.Exp)

            h_ps = ps_h.tile([C, G], f32)
            r_ps = ps_ro.tile([M, G], f32, tag="ro")
            for cc in range(NCH):
                k = CH_B[cc + 1] - CH_B[cc]
                eslice = ex_sb[:k, cc * G:(cc + 1) * G]
                nc.tensor.matmul(h_ps, lhsT=b2_sb[:k, cc * C:cc * C + C],
                                 rhs=eslice, start=(cc == 0), stop=(cc == NCH - 1))
                nc.tensor.matmul(r_ps, lhsT=ones_sb[:k, cc * M:(cc + 1) * M],
                                 rhs=eslice, start=(cc == 0), stop=(cc == NCH - 1))
            rec = rr_pool.tile([M, G], f32, tag="rec")
            nc.vector.reciprocal_approx_fast(rec, r_ps)
            rec_bf = rr_pool.tile([M, G], bf16, tag="recbf")
            nc.vector.tensor_copy(rec_bf, rec)
            rrep_ps = ps_ro.tile([C, G], f32, tag="ro")
            nc.tensor.matmul(rrep_ps, lhsT=erep_sb, rhs=rec_bf, start=True, stop=True)
            rrep = rr_pool.tile([C, G], f32, tag="rrep")
            nc.vector.tensor_copy(rrep, rrep_ps)
            hn = rr_pool.tile([C, G], bf16, tag="hn")
            nc.vector.tensor_mul(hn, h_ps, rrep)
            o_ps = ps_ro.tile([C, G], f32, tag="ro")
            nc.tensor.matmul(o_ps, lhsT=wpt_sb, rhs=hn, start=True, stop=True)
            o_sb = outp.tile([C, G], f32)
            nc.vector.tensor_scalar_add(o_sb, o_ps, bpj_sb)
            nc.gpsimd.dma_start(out=out_d[:, t0:t0 + G], in_=o_sb)

    nc.finalize()
    return nc


def _consts(Wq, Wkv, Wproj, bproj, dw_w, pw_w, ln_w, ln_b):
    import ml_dtypes

    bf16 = ml_dtypes.bfloat16
    scale = HD ** -0.5
    Wk, Wv = Wkv[:128], Wkv[128:]
    ct = np.zeros((M * C, C), np.float32)
    for m in range(M):
        ct[m * C:(m + 1) * C] = scale * Wk[16 * m:16 * m + 16].T @ Wq[16 * m:16 * m + 16]
    dwdiag = np.zeros((C, 36 * C), np.float32)
    for lvl, s in enumerate((8, 4, 2, 1)):
        lscale = (s * s) / float(HW)
        taps = dw_w[:, 0].reshape(C, 9) * lscale
        for t in range(9):
            i = lvl * 9 + t
            dwdiag[:, i * C:(i + 1) * C] = np.diag(taps[:, t])
    onesb = np.zeros((C, NCH * M), np.float32)
    for cc in range(NCH):
        c0, c1 = CH_B[cc], CH_B[cc + 1]
        for r in range(c1 - c0):
            onesb[r, cc * M + (c0 + r) // KV] = 1.0
    return {
        "ct": ct,
        "wvt": np.ascontiguousarray(Wv.T),
        "pwt": np.ascontiguousarray(pw_w[:, :, 0, 0].T),
        "wpt": np.ascontiguousarray(Wproj.T).astype(bf16),
        "dwdiag": dwdiag,
        "onesb": onesb.astype(bf16),
        "lnw": ln_w.reshape(C, 1).astype(np.float32),
        "lnb": ln_b.reshape(C, 1).astype(np.float32),
        "bpj": bproj.reshape(C, 1).astype(np.float32),
        "idn": np.eye(C, dtype=np.float32),
        "erep": np.kron(np.eye(M, dtype=np.float32),
                        np.ones((1, HD), np.float32)).astype(bf16),
        "pos": _pos_full(),
    }


def kernel(x, Wq, Wkv, Wproj, bproj, dw_w, pw_w, ln_w, ln_b):
    from concourse.bass_utils import run_bass_kernel_spmd

    if "nc" not in _CACHE:
        _CACHE["nc"] = _build()
    nc = _CACHE["nc"]

    cst = _consts(np.asarray(Wq, np.float32), np.asarray(Wkv, np.float32),
                  np.asarray(Wproj, np.float32), np.asarray(bproj, np.float32),
                  np.asarray(dw_w, np.float32), np.asarray(pw_w, np.float32),
                  np.asarray(ln_w, np.float32), np.asarray(ln_b, np.float32))
    x = np.asarray(x, np.float32)
    in_maps = []
    for b in range(B):
        im = {"x": np.ascontiguousarray(x[b].reshape(C, HW))}
        im.update(cst)
        in_maps.append(im)

    trace = bool(int(os.environ.get("KPROF", "0")))
    res = run_bass_kernel_spmd(nc, in_maps, core_ids=list(range(B)), trace=trace)
    if trace and res.exec_time_ns is not None:
        print(f"HW exec time: {res.exec_time_ns} ns")
    out = np.stack([res.results[b]["out"].reshape(C, Hh, Ww) for b in range(B)])
    return out



# revision 28
# speedup vs baseline: 2.7865x; 1.3395x over previous
"""Trainium2 Bass kernel for nn_Aspp_Attention: ASPP-KV attention over 2D features.

Sharding: pure data-parallel — batch b=8 over 8 NeuronCores, one image per core.
Device dataflow per core (x: (128 c, 16384 hw) f32):
  xp = x + pos  (pos bf16 const, chunked adds on DVE/GpSimd, DMAs spread over queues)
  ASPP pools (hierarchical sums, chunk-overlapped) -> depthwise 3x3 (GpSimd
  per-channel scalar muls) -> pointwise (PE) -> LN -> gelu -> z2 (c,85) bf16
  A = [0.25*Wq_m^T Wk_m] z2 (c,768 bf16, zero-padded)   [weights folded host-side]
  per token group (512) x chunk-pair: scores^T = A_cc^T @ xp (PE bf16),
  exp on ACT (PSUM->SBUF bf16), H += blockdiag(v)^T exp, r128 += ones2^T exp (PE),
  rec = 1/r128 (DVE), hn = H*rec (DVE), out^T = Wproj @ hn + bproj, DMA out (sync q).
"""
import os
from contextlib import ExitStack

import numpy as np

B, C, Hh, Ww = 8, 128, 128, 128
HW = Hh * Ww
M, HD, KV = 8, 16, 85
KVH = M * KV  # 680
CH_B = [0, 128, 256, 384, 512, 640, 680]
NCH = 6
G = 512            # token group
NG = HW // G       # 32

_CACHE = {}


def _pos_full():
    ch = 64
    inv = 1.0 / (10000.0 ** (np.arange(0, ch, 2, dtype=np.float32) / ch))
    px = np.arange(Hh, dtype=np.float32)[:, None] * inv
    ex = np.concatenate([np.sin(px), np.cos(px)], -1).astype(np.float32)  # (128,64)
    pos = np.zeros((C, Hh, Ww), np.float32)
    pos[:64] = ex.T[:, :, None]
    pos[64:] = ex.T[:, None, :]
    return pos.reshape(C, HW)


def _build():
    import concourse.bass as bass
    import concourse.bacc as bacc
    import concourse.tile as tile
    from concourse import mybir

    nc = bacc.Bacc()
    f32 = mybir.dt.float32
    bf16 = mybir.dt.bfloat16
    AF = mybir.ActivationFunctionType
    AX = mybir.AxisListType

    x_d = nc.dram_tensor("x", [C, HW], f32, kind="ExternalInput")
    pos_d = nc.dram_tensor("pos", [C, HW], bf16, kind="ExternalInput")
    ct_d = nc.dram_tensor("ct", [M * C, C], bf16, kind="ExternalInput")      # lhsT for A
    wvt_d = nc.dram_tensor("wvt", [C, C], bf16, kind="ExternalInput")
    pwt_d = nc.dram_tensor("pwt", [C, C], f32, kind="ExternalInput")
    wpt_d = nc.dram_tensor("wpt", [C, C], bf16, kind="ExternalInput")
    dwt_d = nc.dram_tensor("dwt", [C, 36], f32, kind="ExternalInput")  # taps*lscale
    ones_d = nc.dram_tensor("onesb", [C, NCH * C], bf16, kind="ExternalInput")
    lnw_d = nc.dram_tensor("lnw", [C, 1], f32, kind="ExternalInput")
    lnb_d = nc.dram_tensor("lnb", [C, 1], f32, kind="ExternalInput")
    bpj_d = nc.dram_tensor("bpj", [C, 1], f32, kind="ExternalInput")
    idn_d = nc.dram_tensor("idn", [C, C], f32, kind="ExternalInput")
    out_d = nc.dram_tensor("out", [C, HW], f32, kind="ExternalOutput")

    with ExitStack() as ctx:
        tc = ctx.enter_context(tile.TileContext(nc))
        singles = ctx.enter_context(tc.tile_pool(name="singles", bufs=1))
        xpool = ctx.enter_context(tc.tile_pool(name="xp", bufs=1))
        small = ctx.enter_context(tc.tile_pool(name="small", bufs=3))
        exp_pool = ctx.enter_context(tc.tile_pool(name="exp", bufs=2))
        outp = ctx.enter_context(tc.tile_pool(name="outs", bufs=3))
        rr_pool = ctx.enter_context(tc.tile_pool(name="rr", bufs=2))
        ps_sc = ctx.enter_context(tc.tile_pool(name="psS", bufs=2, space="PSUM"))
        ps_h = ctx.enter_context(tc.tile_pool(name="psH", bufs=2, space="PSUM"))
        ps_r = ctx.enter_context(tc.tile_pool(name="psR", bufs=1, space="PSUM"))
        ps_ro = ctx.enter_context(tc.tile_pool(name="psRO", bufs=1, space="PSUM"))

        dmae = [nc.sync, nc.scalar, nc.gpsimd]

        # ---- load constants (vector queue; light prelude use)
        ct_sb = singles.tile([C, M * C], bf16)      # ct_sb[:, m*C:(m+1)*C] = CT_m
        for m in range(M):
            nc.sync.dma_start(out=ct_sb[:, m * C:(m + 1) * C],
                                in_=ct_d[m * C:(m + 1) * C, :])
        wvt_sb = singles.tile([C, C], bf16)
        nc.sync.dma_start(out=wvt_sb, in_=wvt_d[:, :])
        pwt_sb = singles.tile([C, C], f32)
        nc.sync.dma_start(out=pwt_sb, in_=pwt_d[:, :])
        wpt_sb = singles.tile([C, C], bf16)
        nc.sync.dma_start(out=wpt_sb, in_=wpt_d[:, :])
        dwt_sb = singles.tile([C, 36], f32)
        nc.sync.dma_start(out=dwt_sb, in_=dwt_d[:, :])
        ones_sb = singles.tile([C, NCH * C], bf16)
        nc.scalar.dma_start(out=ones_sb, in_=ones_d[:, :])
        lnw_sb = singles.tile([C, 1], f32)
        nc.sync.dma_start(out=lnw_sb, in_=lnw_d[:, :])
        lnb_sb = singles.tile([C, 1], f32)
        nc.sync.dma_start(out=lnb_sb, in_=lnb_d[:, :])
        bpj_sb = singles.tile([C, 1], f32)
        nc.sync.dma_start(out=bpj_sb, in_=bpj_d[:, :])
        idn_sb = singles.tile([C, C], f32)
        nc.scalar.dma_start(out=idn_sb, in_=idn_d[:, :])

        # ---- x + pos -> xp, chunk-overlapped with level-1 pool sums
        xp_sb = xpool.tile([C, HW], bf16)
        s1 = singles.tile([C, Hh, 8], f32)   # sum over w-blocks of 16
        NXC = 8
        xc = HW // NXC
        for i in range(NXC):
            xt = small.tile([C, xc], f32, tag="xin")
            pt = small.tile([C, xc], bf16, tag="pin")
            dmae[i % 3].dma_start(out=xt, in_=x_d[:, i * xc:(i + 1) * xc])
            dmae[(i + 1) % 3].dma_start(out=pt, in_=pos_d[:, i * xc:(i + 1) * xc])
            xps = xp_sb[:, i * xc:(i + 1) * xc]
            nc.vector.tensor_add(xps, xt, pt)
            nc.vector.reduce_sum(
                s1[:, i * 16:(i + 1) * 16, :],
                xps.rearrange("c (h wg wi) -> c h wg wi", wg=8, wi=16), axis=AX.X)

        # ---- remaining pool levels (sums, gpsimd)
        p8 = singles.tile([C, 8, 8], f32)
        nc.vector.reduce_sum(
            p8, s1.rearrange("c (hg hi) wg -> c hg wg hi", hi=16), axis=AX.X)
        p4 = singles.tile([C, 4, 4], f32)
        t44 = singles.tile([C, 8, 4], f32)
        nc.vector.reduce_sum(t44, p8.rearrange("c h (wg wi) -> c h wg wi", wi=2), axis=AX.X)
        nc.vector.reduce_sum(p4, t44.rearrange("c (hg hi) w -> c hg w hi", hi=2), axis=AX.X)
        p2 = singles.tile([C, 2, 2], f32)
        t22 = singles.tile([C, 4, 2], f32)
        nc.vector.reduce_sum(t22, p4.rearrange("c h (wg wi) -> c h wg wi", wi=2), axis=AX.X)
        nc.vector.reduce_sum(p2, t22.rearrange("c (hg hi) w -> c hg w hi", hi=2), axis=AX.X)
        p1 = singles.tile([C, 1, 1], f32)
        t11 = singles.tile([C, 2, 1], f32)
        nc.vector.reduce_sum(t11, p2.rearrange("c h (wg wi) -> c h wg wi", wi=2), axis=AX.X)
        nc.vector.reduce_sum(p1, t11.rearrange("c (hg hi) w -> c hg w hi", hi=2), axis=AX.X)

        # ---- depthwise 3x3 on each level (padded, 9 taps) on gpsimd -> dwcat
        dwcat = singles.tile([C, KV], f32)
        offs = {8: 0, 4: 64, 2: 80, 1: 84}
        for lvl, (s, ps) in enumerate(((8, p8), (4, p4), (2, p2), (1, p1))):
            pad = singles.tile([C, (s + 2) * (s + 2)], f32, tag=f"pad{s}")
            nc.vector.memset(pad, 0.0)
            pad3 = pad.rearrange("c (h w) -> c h w", h=s + 2)
            nc.vector.tensor_copy(pad3[:, 1:s + 1, 1:s + 1], ps)
            o = offs[s]
            acc = dwcat[:, o:o + s * s].rearrange("c (h w) -> c h w", h=s)
            tmp = singles.tile([C, s, s], f32, tag=f"tmp{s}")
            for di in range(3):
                for dj in range(3):
                    t = 3 * di + dj
                    tap = dwt_sb[:, lvl * 9 + t:lvl * 9 + t + 1]
                    src = pad3[:, di:di + s, dj:dj + s]
                    if t == 0:
                        nc.vector.tensor_scalar_mul(acc, src, tap)
                    else:
                        nc.vector.tensor_scalar_mul(tmp, src, tap)
                        nc.vector.tensor_add(acc, acc, tmp)

        # ---- pointwise conv (PE): z1 = PW @ dwcat
        z1_ps = ps_ro.tile([C, KV], f32, tag="ro")
        nc.tensor.matmul(z1_ps, lhsT=pwt_sb, rhs=dwcat, start=True, stop=True)
        z1_sb = singles.tile([C, KV], f32)
        nc.scalar.copy(z1_sb, z1_ps)

        # ---- LN over c: transpose -> stats -> zn -> transpose back -> gelu
        zt_ps = ps_ro.tile([KV, C], f32, tag="ro")
        nc.tensor.transpose(zt_ps, z1_sb, idn_sb)
        zt_sb = singles.tile([KV, C], f32)
        nc.scalar.copy(zt_sb, zt_ps)
        nmu = singles.tile([KV, 1], f32)
        nc.vector.reduce_sum(nmu, zt_sb, axis=AX.X, negate=True)
        nc.vector.tensor_scalar_mul(nmu, nmu, 1.0 / C)
        zc = singles.tile([KV, C], f32)
        nc.vector.tensor_scalar_add(zc, zt_sb, nmu)
        sq = singles.tile([KV, C], f32)
        nc.vector.tensor_mul(sq, zc, zc)
        var = singles.tile([KV, 1], f32)
        nc.vector.reduce_sum(var, sq, axis=AX.X)
        std = singles.tile([KV, 1], f32)
        eps_sb = singles.tile([KV, 1], f32)
        nc.vector.memset(eps_sb, 1e-5)
        nc.scalar.activation(std, var, AF.Sqrt, bias=eps_sb, scale=1.0 / C)
        rstd = singles.tile([KV, 1], f32)
        nc.vector.reciprocal(rstd, std)
        zn = singles.tile([KV, C], f32)
        nc.vector.tensor_scalar_mul(zn, zc, rstd)
        znt_ps = ps_ro.tile([C, KV], f32, tag="ro")
        nc.tensor.transpose(znt_ps, zn, idn_sb[:KV, :KV])
        z2 = singles.tile([C, KV], bf16)
        nc.scalar.activation(z2, znt_ps, AF.Gelu, bias=lnb_sb, scale=lnw_sb)

        # ---- A (c, 768 zero-padded), vkv (85, 128) bf16, b2 blockdiag chunks
        a_sb = singles.tile([C, NCH * C], bf16)
        nc.vector.memset(a_sb[:, KVH:], 0.0)
        for half in range(2):
            a_ps = ps_ro.tile([C, 4 * KV], f32, tag="ro")
            for mi in range(4):
                m = half * 4 + mi
                nc.tensor.matmul(a_ps[:, mi * KV:(mi + 1) * KV],
                                 lhsT=ct_sb[:, m * C:(m + 1) * C], rhs=z2,
                                 start=True, stop=True)
            nc.scalar.copy(a_sb[:, half * 4 * KV:(half + 1) * 4 * KV], a_ps)
        vt_ps = ps_ro.tile([C, KV], f32, tag="ro")
        nc.tensor.matmul(vt_ps, lhsT=wvt_sb, rhs=z2, start=True, stop=True)
        vt_sb = singles.tile([C, KV], f32)
        nc.scalar.copy(vt_sb, vt_ps)
        vkv_ps = ps_ro.tile([KV, C], f32, tag="ro")
        nc.tensor.transpose(vkv_ps, vt_sb, idn_sb)
        vkv_sb = singles.tile([KV, C], bf16)
        nc.scalar.copy(vkv_sb, vkv_ps)

        b2_sb = singles.tile([C, NCH * C], bf16)
        nc.vector.memset(b2_sb, 0.0)
        for m in range(M):
            g0, g1 = KV * m, KV * (m + 1)
            for cchunk in range(NCH):
                c0, c1 = CH_B[cchunk], CH_B[cchunk + 1]
                lo, hi = max(g0, c0), min(g1, c1)
                if lo >= hi:
                    continue
                nc.scalar.dma_start(
                    out=b2_sb[lo - c0:hi - c0,
                              cchunk * C + HD * m: cchunk * C + HD * m + HD],
                    in_=vkv_sb[lo - g0:hi - g0, HD * m:HD * m + HD])

        # ---- main attention loop over token groups, chunk-pair pipelined
        for g in range(NG):
            t0 = g * G
            xg = xp_sb[:, t0:t0 + G]
            h_ps = ps_h.tile([C, G], f32, tag="h")
            r_ps = ps_r.tile([C, G], f32, tag="r")
            ex_sb = exp_pool.tile([C, NCH * G], bf16, tag="ex")
            for pp in range(3):
                s_ps = ps_sc.tile([C, 2 * G], f32, tag="s")
                for j in range(2):
                    cc = 2 * pp + j
                    nc.tensor.matmul(s_ps[:, j * G:(j + 1) * G],
                                     lhsT=a_sb[:, cc * C:(cc + 1) * C], rhs=xg,
                                     start=True, stop=True)
                nc.scalar.activation(ex_sb[:, 2 * pp * G:(2 * pp + 2) * G], s_ps,
                                     AF.Exp)
                for j in range(2):
                    cc = 2 * pp + j
                    k = CH_B[cc + 1] - CH_B[cc]
                    eslice = ex_sb[:k, cc * G:(cc + 1) * G]
                    nc.tensor.matmul(h_ps, lhsT=b2_sb[:k, cc * C:cc * C + C],
                                     rhs=eslice, start=(cc == 0), stop=(cc == NCH - 1))
                    nc.tensor.matmul(r_ps, lhsT=ones_sb[:k, cc * C:(cc + 1) * C],
                                     rhs=eslice, start=(cc == 0), stop=(cc == NCH - 1))
            rec = rr_pool.tile([C, G], f32, tag="rec")
            nc.vector.reciprocal_approx_fast(rec, r_ps)
            hn = rr_pool.tile([C, G], bf16, tag="hn")
            nc.vector.tensor_mul(hn, h_ps, rec)
            o_ps = ps_ro.tile([C, G], f32, tag="ro")
            nc.tensor.matmul(o_ps, lhsT=wpt_sb, rhs=hn, start=True, stop=True)
            o_sb = outp.tile([C, G], f32)
            nc.vector.tensor_scalar_add(o_sb, o_ps, bpj_sb)
            nc.sync.dma_start(out=out_d[:, t0:t0 + G], in_=o_sb)

    nc.finalize()
    return nc


def _consts(Wq, Wkv, Wproj, bproj, dw_w, pw_w, ln_w, ln_b):
    import ml_dtypes

    bf16 = ml_dtypes.bfloat16
    scale = HD ** -0.5
    Wk, Wv = Wkv[:128], Wkv[128:]
    ct = np.zeros((M * C, C), np.float32)
    for m in range(M):
        ct[m * C:(m + 1) * C] = scale * Wk[16 * m:16 * m + 16].T @ Wq[16 * m:16 * m + 16]
    dwt = np.zeros((C, 36), np.float32)
    for lvl, s in enumerate((8, 4, 2, 1)):
        lscale = (s * s) / float(HW)
        dwt[:, lvl * 9:(lvl + 1) * 9] = dw_w[:, 0].reshape(C, 9) * lscale
    # expanded ones: chunk cc row r hits all 16 channels of its head
    onesb = np.zeros((C, NCH * C), np.float32)
    for cc in range(NCH):
        c0, c1 = CH_B[cc], CH_B[cc + 1]
        for r in range(c1 - c0):
            m = (c0 + r) // KV
            onesb[r, cc * C + HD * m: cc * C + HD * (m + 1)] = 1.0
    return {
        "ct": ct.astype(bf16),
        "wvt": np.ascontiguousarray(Wv.T).astype(bf16),
        "pwt": np.ascontiguousarray(pw_w[:, :, 0, 0].T),
        "wpt": np.ascontiguousarray(Wproj.T).astype(bf16),
        "dwt": dwt,
        "onesb": onesb.astype(bf16),
        "lnw": ln_w.reshape(C, 1).astype(np.float32),
        "lnb": ln_b.reshape(C, 1).astype(np.float32),
        "bpj": bproj.reshape(C, 1).astype(np.float32),
        "idn": np.eye(C, dtype=np.float32),
        "pos": _pos_full().astype(bf16),
    }


def kernel(x, Wq, Wkv, Wproj, bproj, dw_w, pw_w, ln_w, ln_b):
    from concourse.bass_utils import run_bass_kernel_spmd

    if "nc" not in _CACHE:
        _CACHE["nc"] = _build()
    nc = _CACHE["nc"]

    cst = _consts(np.asarray(Wq, np.float32), np.asarray(Wkv, np.float32),
                  np.asarray(Wproj, np.float32), np.asarray(bproj, np.float32),
                  np.asarray(dw_w, np.float32), np.asarray(pw_w, np.float32),
                  np.asarray(ln_w, np.float32), np.asarray(ln_b, np.float32))
    x = np.asarray(x, np.float32)
    in_maps = []
    for b in range(B):
        im = {"x": np.ascontiguousarray(x[b].reshape(C, HW))}
        im.update(cst)
        in_maps.append(im)

    trace = bool(int(os.environ.get("KPROF", "0")))
    res = run_bass_kernel_spmd(nc, in_maps, core_ids=list(range(B)), trace=trace)
    if trace and res.exec_time_ns is not None:
        print(f"HW exec time: {res.exec_time_ns} ns")
    out = np.stack([res.results[b]["out"].reshape(C, Hh, Ww) for b in range(B)])
    return out


# revision 30
# speedup vs baseline: 2.8036x; 1.0062x over previous
"""Trainium2 Bass kernel for nn_Aspp_Attention: ASPP-KV attention over 2D features.

Sharding: pure data-parallel — batch b=8 over 8 NeuronCores, one image per core.
Device dataflow per core (x: (128 c, 16384 hw) f32):
  pools run on raw x (pos pool-sums folded host-side); depthwise3x3+pointwise
  fused into 36 accumulating PE matmuls (per-level scale dropped: LN-invariant);
  LN -> gelu -> z2 (c,85) bf16; A = [0.25*Wq_m^T Wk_m] z2 (c,768 bf16, padded).
  xp = x + pos adds (DVE) stream into the first loop iterations.
  Loop per token group (512), chunk-pairs: scores^T = A_cc^T @ xp (PE bf16),
  exp on ACT (PSUM->SBUF bf16), H += blockdiag(v)^T exp, r128 += ones2^T exp (PE),
  rec = 1/r128, hn = H*rec (DVE); tail (Wproj MM + bias store + DMA) pipelined
  one group behind so the PE never waits on the DVE chain.
"""
import os
from contextlib import ExitStack

import numpy as np

B, C, Hh, Ww = 8, 128, 128, 128
HW = Hh * Ww
M, HD, KV = 8, 16, 85
KVH = M * KV  # 680
CH_B = [0, 128, 256, 384, 512, 640, 680]
NCH = 6
G = 512            # token group
NG = HW // G       # 32

_CACHE = {}


def _pos_full():
    ch = 64
    inv = 1.0 / (10000.0 ** (np.arange(0, ch, 2, dtype=np.float32) / ch))
    px = np.arange(Hh, dtype=np.float32)[:, None] * inv
    ex = np.concatenate([np.sin(px), np.cos(px)], -1).astype(np.float32)  # (128,64)
    pos = np.zeros((C, Hh, Ww), np.float32)
    pos[:64] = ex.T[:, :, None]
    pos[64:] = ex.T[:, None, :]
    return pos.reshape(C, HW)


def _build():
    import concourse.bass as bass
    import concourse.bacc as bacc
    import concourse.tile as tile
    from concourse import mybir

    nc = bacc.Bacc()
    f32 = mybir.dt.float32
    bf16 = mybir.dt.bfloat16
    AF = mybir.ActivationFunctionType
    AX = mybir.AxisListType

    x_d = nc.dram_tensor("x", [C, HW], f32, kind="ExternalInput")
    pos_d = nc.dram_tensor("pos", [C, HW], bf16, kind="ExternalInput")
    ct_d = nc.dram_tensor("ct", [M * C, C], bf16, kind="ExternalInput")  # lhsT for A
    wvt_d = nc.dram_tensor("wvt", [C, C], bf16, kind="ExternalInput")
    pwdw_d = nc.dram_tensor("pwdw", [9 * C, C], bf16, kind="ExternalInput")
    wpt_d = nc.dram_tensor("wpt", [C, C], bf16, kind="ExternalInput")
    pp_d = nc.dram_tensor("pospool", [C, KV], f32, kind="ExternalInput")
    ones_d = nc.dram_tensor("onesb", [C, NCH * C], bf16, kind="ExternalInput")
    lnw_d = nc.dram_tensor("lnw", [C, 1], f32, kind="ExternalInput")
    lnb_d = nc.dram_tensor("lnb", [C, 1], f32, kind="ExternalInput")
    bpj_d = nc.dram_tensor("bpj", [C, 1], f32, kind="ExternalInput")
    idn_d = nc.dram_tensor("idn", [C, C], f32, kind="ExternalInput")
    out_d = nc.dram_tensor("out", [C, HW], f32, kind="ExternalOutput")

    with ExitStack() as ctx:
        tc = ctx.enter_context(tile.TileContext(nc))
        singles = ctx.enter_context(tc.tile_pool(name="singles", bufs=1))
        xpool = ctx.enter_context(tc.tile_pool(name="xp", bufs=1))
        exp_pool = ctx.enter_context(tc.tile_pool(name="exp", bufs=2))
        outp = ctx.enter_context(tc.tile_pool(name="outs", bufs=3))
        rr_pool = ctx.enter_context(tc.tile_pool(name="rr", bufs=2))
        ps_sc = ctx.enter_context(tc.tile_pool(name="psS", bufs=2, space="PSUM"))
        ps_h = ctx.enter_context(tc.tile_pool(name="psH", bufs=2, space="PSUM"))
        ps_r = ctx.enter_context(tc.tile_pool(name="psR", bufs=1, space="PSUM"))
        ps_ro = ctx.enter_context(tc.tile_pool(name="psRO", bufs=1, space="PSUM"))

        dmae = [nc.sync, nc.scalar, nc.gpsimd]

        # ---- load constants
        ct_sb = singles.tile([C, M * C], bf16)      # ct_sb[:, m*C:(m+1)*C] = CT_m
        for m in range(M):
            nc.sync.dma_start(out=ct_sb[:, m * C:(m + 1) * C],
                              in_=ct_d[m * C:(m + 1) * C, :])
        wvt_sb = singles.tile([C, C], bf16)
        nc.sync.dma_start(out=wvt_sb, in_=wvt_d[:, :])
        pwdw_sb = singles.tile([C, 9 * C], bf16)    # pwdw_sb[:, t*C:] = lhsT_t
        for t in range(9):
            nc.scalar.dma_start(out=pwdw_sb[:, t * C:(t + 1) * C],
                                in_=pwdw_d[t * C:(t + 1) * C, :])
        wpt_sb = singles.tile([C, C], bf16)
        nc.sync.dma_start(out=wpt_sb, in_=wpt_d[:, :])
        pp_sb = singles.tile([C, KV], f32)
        nc.sync.dma_start(out=pp_sb, in_=pp_d[:, :])
        ones_sb = singles.tile([C, NCH * C], bf16)
        nc.scalar.dma_start(out=ones_sb, in_=ones_d[:, :])
        lnw_sb = singles.tile([C, 1], f32)
        nc.sync.dma_start(out=lnw_sb, in_=lnw_d[:, :])
        lnb_sb = singles.tile([C, 1], f32)
        nc.sync.dma_start(out=lnb_sb, in_=lnb_d[:, :])
        bpj_sb = singles.tile([C, 1], f32)
        nc.sync.dma_start(out=bpj_sb, in_=bpj_d[:, :])
        idn_sb = singles.tile([C, C], f32)
        nc.scalar.dma_start(out=idn_sb, in_=idn_d[:, :])

        # ---- stream x/pos in; level-1 pool sums on raw x chunks
        NXC = 8
        xc = HW // NXC
        s1 = singles.tile([C, Hh, 8], f32)   # x summed over w-blocks of 16
        xst = []
        pst = []
        for i in range(NXC):
            xt = singles.tile([C, xc], f32, tag=f"xin{i}")
            pt = singles.tile([C, xc], bf16, tag=f"pin{i}")
            dmae[i % 3].dma_start(out=xt, in_=x_d[:, i * xc:(i + 1) * xc])
            dmae[(i + 1) % 3].dma_start(out=pt, in_=pos_d[:, i * xc:(i + 1) * xc])
            xst.append(xt)
            pst.append(pt)
            nc.vector.reduce_sum(
                s1[:, i * 16:(i + 1) * 16, :],
                xt.rearrange("c (h wg wi) -> c h wg wi", wg=8, wi=16), axis=AX.X)

        # ---- remaining pool levels (sums) + host-folded pos pool sums
        p8 = singles.tile([C, 8, 8], f32)
        nc.vector.reduce_sum(
            p8, s1.rearrange("c (hg hi) wg -> c hg wg hi", hi=16), axis=AX.X)
        p4 = singles.tile([C, 4, 4], f32)
        t44 = singles.tile([C, 8, 4], f32)
        nc.vector.reduce_sum(t44, p8.rearrange("c h (wg wi) -> c h wg wi", wi=2), axis=AX.X)
        nc.vector.reduce_sum(p4, t44.rearrange("c (hg hi) w -> c hg w hi", hi=2), axis=AX.X)
        p2 = singles.tile([C, 2, 2], f32)
        t22 = singles.tile([C, 4, 2], f32)
        nc.vector.reduce_sum(t22, p4.rearrange("c h (wg wi) -> c h wg wi", wi=2), axis=AX.X)
        nc.vector.reduce_sum(p2, t22.rearrange("c (hg hi) w -> c hg w hi", hi=2), axis=AX.X)
        p1 = singles.tile([C, 1, 1], f32)
        t11 = singles.tile([C, 2, 1], f32)
        nc.vector.reduce_sum(t11, p2.rearrange("c h (wg wi) -> c h wg wi", wi=2), axis=AX.X)
        nc.vector.reduce_sum(p1, t11.rearrange("c (hg hi) w -> c hg w hi", hi=2), axis=AX.X)
        offs = {8: 0, 4: 64, 2: 80, 1: 84}
        for s, ps in ((8, p8), (4, p4), (2, p2), (1, p1)):
            o = offs[s]
            psl = pp_sb[:, o:o + s * s].rearrange("c (h w) -> c h w", h=s)
            nc.vector.tensor_add(ps, ps, psl)

        # ---- fused depthwise+pointwise: z1 = sum_t PWdiag(tap_t) @ pad_shift_t
        # (per-level 1/blk scale dropped -- LN normalizes it out)
        z1_ps = ps_ro.tile([C, KV], f32, tag="ro")
        for lvl, (s, ps) in enumerate(((8, p8), (4, p4), (2, p2), (1, p1))):
            pad = singles.tile([C, (s + 2) * (s + 2)], bf16, tag=f"pad{s}")
            nc.vector.memset(pad, 0.0)
            pad3 = pad.rearrange("c (h w) -> c h w", h=s + 2)
            nc.vector.tensor_copy(pad3[:, 1:s + 1, 1:s + 1], ps)
            o = offs[s]
            dst = z1_ps[:, o:o + s * s].rearrange("c (h w) -> c h w", h=s)
            for di in range(3):
                for dj in range(3):
                    t = 3 * di + dj
                    nc.tensor.matmul(dst, lhsT=pwdw_sb[:, t * C:(t + 1) * C],
                                     rhs=pad3[:, di:di + s, dj:dj + s],
                                     start=(t == 0), stop=(t == 8))
        z1_sb = singles.tile([C, KV], f32)
        nc.scalar.copy(z1_sb, z1_ps)

        # ---- LN over c: transpose -> stats -> zn -> transpose back -> gelu
        zt_ps = ps_ro.tile([KV, C], f32, tag="ro")
        nc.tensor.transpose(zt_ps, z1_sb, idn_sb)
        zt_sb = singles.tile([KV, C], f32)
        nc.scalar.copy(zt_sb, zt_ps)
        nmu = singles.tile([KV, 1], f32)
        nc.vector.reduce_sum(nmu, zt_sb, axis=AX.X, negate=True)
        nc.vector.tensor_scalar_mul(nmu, nmu, 1.0 / C)
        zc = singles.tile([KV, C], f32)
        nc.vector.tensor_scalar_add(zc, zt_sb, nmu)
        sq = singles.tile([KV, C], f32)
        nc.vector.tensor_mul(sq, zc, zc)
        var = singles.tile([KV, 1], f32)
        nc.vector.reduce_sum(var, sq, axis=AX.X)
        std = singles.tile([KV, 1], f32)
        eps_sb = singles.tile([KV, 1], f32)
        nc.vector.memset(eps_sb, 1e-5)
        nc.scalar.activation(std, var, AF.Sqrt, bias=eps_sb, scale=1.0 / C)
        rstd = singles.tile([KV, 1], f32)
        nc.vector.reciprocal(rstd, std)
        zn = singles.tile([KV, C], f32)
        nc.vector.tensor_scalar_mul(zn, zc, rstd)
        znt_ps = ps_ro.tile([C, KV], f32, tag="ro")
        nc.tensor.transpose(znt_ps, zn, idn_sb[:KV, :KV])
        z2 = singles.tile([C, KV], bf16)
        nc.scalar.activation(z2, znt_ps, AF.Gelu, bias=lnb_sb, scale=lnw_sb)

        # ---- A (c, 768 zero-padded), vkv (85, 128) bf16, b2 blockdiag chunks
        a_sb = singles.tile([C, NCH * C], bf16)
        nc.vector.memset(a_sb[:, KVH:], 0.0)
        for half in range(2):
            a_ps = ps_ro.tile([C, 4 * KV], f32, tag="ro")
            for mi in range(4):
                m = half * 4 + mi
                nc.tensor.matmul(a_ps[:, mi * KV:(mi + 1) * KV],
                                 lhsT=ct_sb[:, m * C:(m + 1) * C], rhs=z2,
                                 start=True, stop=True)
            nc.scalar.copy(a_sb[:, half * 4 * KV:(half + 1) * 4 * KV], a_ps)
        vt_ps = ps_ro.tile([C, KV], f32, tag="ro")
        nc.tensor.matmul(vt_ps, lhsT=wvt_sb, rhs=z2, start=True, stop=True)
        vt_sb = singles.tile([C, KV], f32)
        nc.scalar.copy(vt_sb, vt_ps)
        vkv_ps = ps_ro.tile([KV, C], f32, tag="ro")
        nc.tensor.transpose(vkv_ps, vt_sb, idn_sb)
        vkv_sb = singles.tile([KV, C], bf16)
        nc.scalar.copy(vkv_sb, vkv_ps)

        b2_sb = singles.tile([C, NCH * C], bf16)
        nc.vector.memset(b2_sb, 0.0)
        for m in range(M):
            g0, g1 = KV * m, KV * (m + 1)
            for cchunk in range(NCH):
                c0, c1 = CH_B[cchunk], CH_B[cchunk + 1]
                lo, hi = max(g0, c0), min(g1, c1)
                if lo >= hi:
                    continue
                nc.scalar.dma_start(
                    out=b2_sb[lo - c0:hi - c0,
                              cchunk * C + HD * m: cchunk * C + HD * m + HD],
                    in_=vkv_sb[lo - g0:hi - g0, HD * m:HD * m + HD])

        # ---- main attention loop; xp adds streamed into iterations 0..15,
        # Wproj/store tail pipelined one group behind
        xp_sb = xpool.tile([C, HW], bf16)
        GH = 1024
        pend = [None]

        def emit_tail():
            hn_, tt = pend[0]
            o_ps = ps_ro.tile([C, G], f32, tag="ro")
            nc.tensor.matmul(o_ps, lhsT=wpt_sb, rhs=hn_, start=True, stop=True)
            o_sb = outp.tile([C, G], f32)
            nc.scalar.activation(o_sb, o_ps, AF.Identity, bias=bpj_sb)
            nc.sync.dma_start(out=out_d[:, tt:tt + G], in_=o_sb)
            pend[0] = None

        for g in range(NG):
            if g < 16:
                # xp half-chunk g = x + pos over tokens [g*1024, (g+1)*1024)
                i, hh = g // 2, (g % 2) * GH
                nc.vector.tensor_add(xp_sb[:, g * GH:(g + 1) * GH],
                                     xst[i][:, hh:hh + GH], pst[i][:, hh:hh + GH])
            t0 = g * G
            xg = xp_sb[:, t0:t0 + G]
            h_ps = ps_h.tile([C, G], f32, tag="h")
            r_ps = ps_r.tile([C, G], f32, tag="r")
            ex_sb = exp_pool.tile([C, NCH * G], bf16, tag="ex")
            for pp in range(3):
                s_ps = ps_sc.tile([C, 2 * G], f32, tag="s")
                for j in range(2):
                    cc = 2 * pp + j
                    nc.tensor.matmul(s_ps[:, j * G:(j + 1) * G],
                                     lhsT=a_sb[:, cc * C:(cc + 1) * C], rhs=xg,
                                     start=True, stop=True)
                if pp == 1 and pend[0] is not None:
                    emit_tail()
                nc.scalar.activation(ex_sb[:, 2 * pp * G:(2 * pp + 2) * G], s_ps,
                                     AF.Exp)
                for j in range(2):
                    cc = 2 * pp + j
                    k = CH_B[cc + 1] - CH_B[cc]
                    eslice = ex_sb[:k, cc * G:(cc + 1) * G]
                    nc.tensor.matmul(h_ps, lhsT=b2_sb[:k, cc * C:cc * C + C],
                                     rhs=eslice, start=(cc == 0), stop=(cc == NCH - 1))
                    nc.tensor.matmul(r_ps, lhsT=ones_sb[:k, cc * C:(cc + 1) * C],
                                     rhs=eslice, start=(cc == 0), stop=(cc == NCH - 1))
            rec = rr_pool.tile([C, G], f32, tag="rec")
            nc.vector.reciprocal_approx_fast(rec, r_ps)
            hn = rr_pool.tile([C, G], bf16, tag="hn")
            nc.vector.tensor_mul(hn, h_ps, rec)
            pend[0] = (hn, t0)
        emit_tail()

    nc.finalize()
    return nc


def _consts(Wq, Wkv, Wproj, bproj, dw_w, pw_w, ln_w, ln_b):
    import ml_dtypes

    bf16 = ml_dtypes.bfloat16
    scale = HD ** -0.5
    Wk, Wv = Wkv[:128], Wkv[128:]
    ct = np.zeros((M * C, C), np.float32)
    for m in range(M):
        ct[m * C:(m + 1) * C] = scale * Wk[16 * m:16 * m + 16].T @ Wq[16 * m:16 * m + 16]
    # fused pointwise*diag(dw tap t), transposed for lhsT
    pw = pw_w[:, :, 0, 0]                      # (C out, C in)
    taps = dw_w[:, 0].reshape(C, 9)            # (C, 9)
    pwdw = np.zeros((9 * C, C), np.float32)
    for t in range(9):
        pwdw[t * C:(t + 1) * C] = pw.T * taps[:, t:t + 1]
    # pos pool sums per level, level-major like dwcat
    pos = _pos_full().reshape(C, Hh, Ww)
    pospool = np.zeros((C, KV), np.float32)
    offs = {8: 0, 4: 64, 2: 80, 1: 84}
    for s in (8, 4, 2, 1):
        blk = Hh // s
        psum = pos.reshape(C, s, blk, s, blk).sum((2, 4))
        pospool[:, offs[s]:offs[s] + s * s] = psum.reshape(C, s * s)
    # expanded ones: chunk cc row r hits all 16 channels of its head
    onesb = np.zeros((C, NCH * C), np.float32)
    for cc in range(NCH):
        c0, c1 = CH_B[cc], CH_B[cc + 1]
        for r in range(c1 - c0):
            m = (c0 + r) // KV
            onesb[r, cc * C + HD * m: cc * C + HD * (m + 1)] = 1.0
    return {
        "ct": ct.astype(bf16),
        "wvt": np.ascontiguousarray(Wv.T).astype(bf16),
        "pwdw": pwdw.astype(bf16),
        "wpt": np.ascontiguousarray(Wproj.T).astype(bf16),
        "pospool": pospool,
        "onesb": onesb.astype(bf16),
        "lnw": ln_w.reshape(C, 1).astype(np.float32),
        "lnb": ln_b.reshape(C, 1).astype(np.float32),
        "bpj": bproj.reshape(C, 1).astype(np.float32),
        "idn": np.eye(C, dtype=np.float32),
        "pos": _pos_full().astype(bf16),
    }


def kernel(x, Wq, Wkv, Wproj, bproj, dw_w, pw_w, ln_w, ln_b):
    from concourse.bass_utils import run_bass_kernel_spmd

    if "nc" not in _CACHE:
        _CACHE["nc"] = _build()
    nc = _CACHE["nc"]

    cst = _consts(np.asarray(Wq, np.float32), np.asarray(Wkv, np.float32),
                  np.asarray(Wproj, np.float32), np.asarray(bproj, np.float32),
                  np.asarray(dw_w, np.float32), np.asarray(pw_w, np.float32),
                  np.asarray(ln_w, np.float32), np.asarray(ln_b, np.float32))
    x = np.asarray(x, np.float32)
    in_maps = []
    for b in range(B):
        im = {"x": np.ascontiguousarray(x[b].reshape(C, HW))}
        im.update(cst)
        in_maps.append(im)

    trace = bool(int(os.environ.get("KPROF", "0")))
    res = run_bass_kernel_spmd(nc, in_maps, core_ids=list(range(B)), trace=trace)
    if trace and res.exec_time_ns is not None:
        print(f"HW exec time: {res.exec_time_ns} ns")
    out = np.stack([res.results[b]["out"].reshape(C, Hh, Ww) for b in range(B)])
    return out
